# revision 2
# baseline (speedup 1.0000x reference)
"""HardLSTMLayer Trainium2 kernel.

Strategy:
  - Host-side prep: pre-transpose/scale weights, fold hard-sigmoid affine
    (0.2*x+0.5) into w_ih/w_hh rows for i,f,o gates + bias, cast to fp16,
    pre-transpose x into [block, k, part, t, b] layout.
  - Phase A (on device): xg'[t] = scaled_w_ih @ x[t]^T + bias for all t,
    batched 16 timesteps per matmul block (N=512), stored to DRAM scratch
    in per-step [128, 512] fp16 tiles (col = m*32 + b).
  - Recurrence: 2000 steps; per step 64 weight-stationary matmuls
    (lhsT = w_hh^T fp16 tile [128,128], rhs = h^T chunk [128,32]) into two
    PSUM banks (i,f | g,o), then clamp/elementwise on DVE+GPSIMD, h^T fp16
    written straight to DRAM (host transposes at the end).
  - All 8 cores run the identical replicated program (the recurrence is
    sequential and weight-load bound, so batch/T sharding cannot shorten
    the critical path; core 0's output is used).
"""
import time
from contextlib import ExitStack

import numpy as np

T, B, I, H = 2000, 32, 512, 512
TB = 16          # timesteps per phase-A block
NBLK = T // TB   # 125
U = 8            # recurrence unroll per hardware loop iteration
N_CORES = 8

_built = {}


def _build_module():
    import concourse.bass as bass
    import concourse.tile as tile
    from concourse import bacc, mybir
    from concourse.bass import ds

    f16 = mybir.dt.float16
    f32 = mybir.dt.float32
    Alu = mybir.AluOpType

    nc = bacc.Bacc("TRN2", target_bir_lowering=False, debug=False,
                   num_devices=N_CORES)

    xT = nc.dram_tensor("xT", [NBLK, 4, 128, TB, 32], f16,
                        kind="ExternalInput").ap()
    wihT = nc.dram_tensor("wihT", [16, 4, 128, 128], f16,
                          kind="ExternalInput").ap()
    whhT = nc.dram_tensor("whhT", [16, 4, 128, 128], f16,
                          kind="ExternalInput").ap()
    bias2 = nc.dram_tensor("bias2", [1, 2048], f16, kind="ExternalInput").ap()
    h0T = nc.dram_tensor("h0T", [128, 128], f16, kind="ExternalInput").ap()
    c0T = nc.dram_tensor("c0T", [128, 128], f32, kind="ExternalInput").ap()
    out_t = nc.dram_tensor("out_t", [T, 128, 128], f16,
                           kind="ExternalOutput").ap()
    cT_t = nc.dram_tensor("cT_t", [128, 128], f32, kind="ExternalOutput").ap()
    xgp = nc.dram_tensor("xgp", [T, 128, 512], f16).ap()

    with tile.TileContext(nc) as tc, ExitStack() as ctx:
        consts = ctx.enter_context(tc.tile_pool(name="consts", bufs=1))
        w_sb = consts.tile([128, 16, 4, 128], f16)    # whhT tiles (K,m,k,M)
        wi_sb = consts.tile([128, 16, 4, 128], f16)   # wihT tiles
        ones_sb = consts.tile([1, TB * 32], f16)
        bias_sb = consts.tile([1, 2048], f16)
        h_sb = consts.tile([128, 128], f16)           # h^T state
        c_sb = consts.tile([128, 128], f32)           # c^T state

        nc.sync.dma_start(w_sb[:], whhT.rearrange("m k p q -> p m k q"))
        nc.sync.dma_start(wi_sb[:], wihT.rearrange("m k p q -> p m k q"))
        nc.sync.dma_start(bias_sb[:], bias2[:])
        nc.sync.dma_start(h_sb[:], h0T[:])
        nc.sync.dma_start(c_sb[:], c0T[:])
        nc.vector.memset(ones_sb[:], 1.0)

        # ---------------- Phase A: input projection ----------------
        with tc.tile_pool(name="xa", bufs=3) as xa_pool, \
             tc.tile_pool(name="biga", bufs=2) as big_pool, \
             tc.tile_pool(name="psa", bufs=4, space="PSUM") as psa_pool:
            with tc.For_i(0, NBLK, 1) as blk:
                xc = xa_pool.tile([128, 4, TB * 32], f16)
                nc.sync.dma_start(
                    xc[:], xT[ds(blk, 1)].rearrange("a k p t b -> p k (a t b)"))
                big = big_pool.tile([128, 16, TB * 32], f16)
                for m in range(16):
                    ps = psa_pool.tile([128, TB * 32], f32)
                    for k in range(4):
                        nc.tensor.matmul(ps[:], wi_sb[:, m, k, :], xc[:, k, :],
                                         start=(k == 0), stop=False,
                                         skip_group_check=True)
                    nc.tensor.matmul(ps[:], bias_sb[:, m * 128:(m + 1) * 128],
                                     ones_sb[:], start=False, stop=True,
                                     skip_group_check=True)
                    if m % 2 == 0:
                        nc.scalar.copy(big[:, m, :], ps[:])
                    else:
                        nc.vector.tensor_copy(big[:, m, :], ps[:])
                for tl in range(TB):
                    nc.gpsimd.dma_start(
                        xgp[ds(blk * TB + tl, 1)].rearrange(
                            "a p (m b) -> p m (a b)", m=16),
                        big[:, :, tl * 32:(tl + 1) * 32])

        tc.strict_bb_all_engine_barrier()

        # ---------------- Recurrence ----------------
        with tc.tile_pool(name="xg", bufs=8) as xg_pool, \
             tc.tile_pool(name="ew", bufs=2) as ew_pool, \
             tc.tile_pool(name="psifr", bufs=2, space="PSUM") as psif_pool, \
             tc.tile_pool(name="psgor", bufs=2, space="PSUM") as psgo_pool:
            with tc.For_i(0, T, U, hint_engines=(mybir.EngineType.PE,)) as t0:
                for u in range(U):
                    tt = t0 + u
                    xg = xg_pool.tile([128, 512], f16)
                    nc.sync.dma_start(
                        xg[:], xgp[ds(tt, 1)].rearrange("a p c -> p (a c)"))
                    ps_if = psif_pool.tile([128, 256], f32)
                    ps_go = psgo_pool.tile([128, 256], f32)
                    for m in range(16):
                        dst = (ps_if if m < 8 else ps_go)
                        col = (m % 8) * 32
                        for k in range(4):
                            nc.tensor.matmul(
                                dst[:, col:col + 32], w_sb[:, m, k, :],
                                h_sb[:, k * 32:(k + 1) * 32],
                                start=(k == 0), stop=(k == 3),
                                skip_group_check=True)
                    pre_if = ew_pool.tile([128, 256], f16)
                    nc.vector.tensor_add(pre_if[:], ps_if[:], xg[:, 0:256])
                    pre_go = ew_pool.tile([128, 256], f16)
                    nc.vector.tensor_add(pre_go[:], ps_go[:], xg[:, 256:512])
                    s_if = ew_pool.tile([128, 256], f16)
                    nc.vector.tensor_scalar(s_if[:], pre_if[:], 1.0, 0.0,
                                            Alu.min, Alu.max)
                    gg = ew_pool.tile([128, 128], f16)
                    nc.vector.tensor_scalar(gg[:], pre_go[:, 0:128], 1.0, -1.0,
                                            Alu.min, Alu.max)
                    s_o = ew_pool.tile([128, 128], f16)
                    nc.gpsimd.tensor_scalar(s_o[:], pre_go[:, 128:256], 1.0,
                                            0.0, Alu.min, Alu.max)
                    ig = ew_pool.tile([128, 128], f32)
                    nc.gpsimd.tensor_mul(ig[:], s_if[:, 0:128], gg[:])
                    fc = ew_pool.tile([128, 128], f32)
                    nc.vector.tensor_mul(fc[:], s_if[:, 128:256], c_sb[:])
                    nc.vector.tensor_add(c_sb[:], fc[:], ig[:])
                    cc = ew_pool.tile([128, 128], f16)
                    nc.vector.tensor_scalar(cc[:], c_sb[:], 1.0, -1.0,
                                            Alu.min, Alu.max)
                    nc.vector.tensor_mul(h_sb[:], s_o[:], cc[:])
                    nc.gpsimd.dma_start(
                        out_t[ds(tt, 1)].rearrange("a p c -> p (a c)"),
                        h_sb[:])

        nc.sync.dma_start(cT_t[:], c_sb[:])

    nc.compile()
    return nc


def _prep_inputs(x, h0, c0, w_ih, w_hh, b_ih, b_hh):
    scale = np.full((4 * H, 1), 0.2, np.float32)
    scale[2 * H:3 * H] = 1.0  # g gate unscaled
    b = (b_ih + b_hh).astype(np.float32)
    b2 = b * scale[:, 0]
    b2[:2 * H] += 0.5
    b2[3 * H:] += 0.5

    def tiles_T(w):  # [2048, 512] -> [16, 4, 128, 128] = block transposed
        return np.ascontiguousarray(
            w.reshape(16, 128, 4, 128).transpose(0, 2, 3, 1)).astype(np.float16)

    wihT = tiles_T(w_ih.astype(np.float32) * scale)
    whhT = tiles_T(w_hh.astype(np.float32) * scale)
    xT = np.ascontiguousarray(
        x.reshape(NBLK, TB, B, 4, 128).transpose(0, 3, 4, 1, 2)
    ).astype(np.float16)
    h0T = np.ascontiguousarray(
        h0.reshape(B, 4, 128).transpose(2, 1, 0).reshape(128, 128)
    ).astype(np.float16)
    c0T = np.ascontiguousarray(
        c0.reshape(B, 4, 128).transpose(2, 1, 0).reshape(128, 128)
    ).astype(np.float32)
    return {
        "xT": xT, "wihT": wihT, "whhT": whhT,
        "bias2": b2.reshape(1, 2048).astype(np.float16),
        "h0T": h0T, "c0T": c0T,
    }


def kernel(x, h0, c0, w_ih, w_hh, b_ih, b_hh):
    from concourse.bass_utils import run_bass_kernel_spmd

    x = np.asarray(x, np.float32)
    in_map = _prep_inputs(np.asarray(x, np.float32), np.asarray(h0, np.float32),
                          np.asarray(c0, np.float32),
                          np.asarray(w_ih, np.float32),
                          np.asarray(w_hh, np.float32),
                          np.asarray(b_ih, np.float32),
                          np.asarray(b_hh, np.float32))
    if "nc" not in _built:
        _built["nc"] = _build_module()
    nc = _built["nc"]
    res = run_bass_kernel_spmd(nc, [in_map] * N_CORES, list(range(N_CORES)))
    r0 = res.results[0]
    out_t = r0["out_t"]  # [T, 128, 128] fp16, [t, p, hsub*32+b]
    out = np.ascontiguousarray(
        out_t.reshape(T, 128, 4, 32).transpose(0, 3, 2, 1).reshape(T, B, H)
    ).astype(np.float32)
    cT = np.ascontiguousarray(
        r0["cT_t"].reshape(128, 4, 32).transpose(2, 1, 0).reshape(B, H)
    ).astype(np.float32)
    hT = out[-1].copy()
    return out, hT, cT
